# revision 21
# baseline (speedup 1.0000x reference)
"""Multi-head attention (B=1, S=4096, D=768, H=12, Dh=64) on 8 NeuronCores.

Sharding: 4 head-groups (3 heads each) x 2 query-halves (2048 rows each).
Core c = (g, half) computes, for its 3 heads and its 2048 query rows, the
partial output O_g @ W_o[g_slice] (shape [2048, 768]). Host sums the 4
group partials per half and adds b_o.

All device data is 2-byte: q/k/W_q/W_k/W_o in fp16 (10-bit mantissa — the
score error this introduces is ~1e-3 relative), v/W_v and the exp output
in bf16 (exp(s) reaches ~6e4, too close to the fp16 max, so the P^T/V
path needs bf16 range). PSUM accumulation is fp32 throughout.

Per-core pipeline (layouts chosen so nothing is ever transposed on PE):
  1. q/k/v arrive transposed into [demb-chunk, seq] staging via DMA-xbar
     transpose straight from DRAM (2-byte dtypes; 512-row groups), then:
       qT/kT [hd, seq]  (lhsT = W chunk, rhs = staging, fp16)
       V     [seq, hd]  (lhsT = staging, rhs = W_v, bf16)
     Biases fold in as K=1 accumulation matmuls.
  2. Attention per (head, 512-query-chunk), key blocks in pairs:
       S^T = K_h @ Q_h^T         (2 fp16 matmuls into one 2-bank PSUM tile)
       P^T = exp(S^T/8) -> bf16  (one ACT op per 1024 columns; no
                                  max-subtraction: scores here are ~|11|)
       O_un^T += V_ext^T @ P^T   (bf16; V_ext has a ones column so row 64
                                  accumulates Z = sum exp)
     Emission is a conveyor: PV trails S^T/exp by two steps so the PE
     stream never blocks on ACT; epilogues and the final projections ride
     the conveyor as deferred closures.
     Overlap with the input DMA: the first two (h=0/1, qc=0) iterations
     run interleaved with the k/v loading loop, and head 2's exps are
     buffered to SBUF during it (its PV replays right after).
     Epilogue: invZ = 1/Z (DVE), partition-broadcast on GPSIMD, then
     O^T = O_un^T * invZ_bcast (DVE).
  3. Final projection out[q, :] = O^T.T @ W_o (fp16), drip-fed between
     attention steps, DMA out.
"""

import numpy as np
import ml_dtypes

import concourse.bass as bass
import concourse.mybir as mybir
import concourse.tile as tile
from concourse import bacc
from concourse.bass_utils import run_bass_kernel_spmd

S = 4096          # sequence length
D = 768           # embedding dim
H = 12            # total heads
DH = 64           # head dim
G = 4             # head groups (cores 2g, 2g+1 share a group)
HG = 3            # heads per group
HD = HG * DH      # 192: per-core projected dim
QS = S // 2       # 2048: per-core query rows
N_CORES = 8
SCALE = 1.0 / 8.0  # INV_TEMP / sqrt(DH)

FP32 = mybir.dt.float32
FP16 = mybir.dt.float16
BF16 = mybir.dt.bfloat16
KB = S // 128     # 32 key blocks
QB = QS // 128    # 16 query blocks
DC = D // 128     # 6 embedding chunks
NG = S // 512     # 8 key groups (xbar staging granularity)

Exp = mybir.ActivationFunctionType.Exp


def build_nc():
    """Build the per-core Bass program (same program on all 8 cores)."""
    nc = bacc.Bacc("TRN2", target_bir_lowering=False, debug=False,
                   num_devices=N_CORES)
    dr = {}
    for name, shape, dt in [
            ("q_in", [QS, D], FP16), ("k_in", [S, D], FP16),
            ("v_in", [S, D], BF16), ("w_q", [D, HD], FP16),
            ("w_k", [D, HD], FP16), ("w_v", [D, HD], BF16),
            ("w_o", [HD, D], FP16), ("b_q", [1, HD], FP16),
            ("b_k", [1, HD], FP16), ("b_v", [1, HD], BF16)]:
        dr[name] = nc.dram_tensor(name, shape, dt,
                                  kind="ExternalInput").ap()
    dr["out"] = nc.dram_tensor("out", [QS, D], FP32,
                               kind="ExternalOutput").ap()

    with tile.TileContext(nc) as tc:
        _body(tc, dr)
    nc.compile()
    return nc


def _body(tc, dr):
    nc = tc.nc
    fp = FP32

    from contextlib import ExitStack
    with ExitStack() as stk:
        const = stk.enter_context(tc.tile_pool(name="const", bufs=1))
        wpool = stk.enter_context(tc.tile_pool(name="wpool", bufs=1))
        big = stk.enter_context(tc.tile_pool(name="big", bufs=1))
        stagep = stk.enter_context(tc.tile_pool(name="stagep", bufs=3))
        vstagep = stk.enter_context(tc.tile_pool(name="vstagep", bufs=3))
        ptp = stk.enter_context(tc.tile_pool(name="ptp", bufs=6))
        pt2p = stk.enter_context(tc.tile_pool(name="pt2p", bufs=16))
        smallp = stk.enter_context(tc.tile_pool(name="smallp", bufs=2))
        outp = stk.enter_context(tc.tile_pool(name="outp", bufs=2))
        # PSUM: 2 one-bank projection slots, 2 two-bank S^T slots,
        # 2 one-bank attention accumulators
        mixp = stk.enter_context(
            tc.tile_pool(name="mixp", bufs=2, space="PSUM"))
        stp = stk.enter_context(
            tc.tile_pool(name="stp", bufs=2, space="PSUM"))
        otp = stk.enter_context(
            tc.tile_pool(name="otp", bufs=2, space="PSUM"))

        # ---- constants + weights, critical path first: the very first
        # PE work is the q0 projection (needs w_q, b_q, ones) ----
        ones_16 = const.tile([1, 512], FP16)
        nc.vector.memset(ones_16, 1.0)
        bq_sb = const.tile([1, HD], FP16)
        nc.sync.dma_start(out=bq_sb, in_=dr["b_q"])
        wq_sb = wpool.tile([128, DC, HD], FP16)
        nc.sync.dma_start(
            out=wq_sb, in_=dr["w_q"].rearrange("(po pi) n -> pi po n",
                                               pi=128))
        wk_sb = wpool.tile([128, DC, HD], FP16)
        nc.sync.dma_start(
            out=wk_sb, in_=dr["w_k"].rearrange("(po pi) n -> pi po n",
                                               pi=128))
        bk_sb = const.tile([1, HD], FP16)
        nc.sync.dma_start(out=bk_sb, in_=dr["b_k"])
        wv_sb = wpool.tile([128, DC, HD], BF16)
        nc.sync.dma_start(
            out=wv_sb, in_=dr["w_v"].rearrange("(po pi) n -> pi po n",
                                               pi=128))
        bv_sb = const.tile([1, HD], BF16)
        nc.sync.dma_start(out=bv_sb, in_=dr["b_v"])
        ones_bf = const.tile([1, 512], BF16)
        nc.vector.memset(ones_bf, 1.0)
        wo_a = wpool.tile([128, D], FP16)
        nc.sync.dma_start(out=wo_a, in_=dr["w_o"][0:128, :])
        wo_b = wpool.tile([64, D], FP16)
        nc.sync.dma_start(out=wo_b, in_=dr["w_o"][128:HD, :])

        # ---- persistent activations ----
        # qT/kT [hd, seq]: heads 0,1 packed in tile A rows 0:64 / 64:128,
        # head 2 in tile B rows 0:64. O^T packed the same way.
        kt_a = big.tile([128, S], FP16)
        kt_b = big.tile([64, S], FP16)
        qt_a = big.tile([128, QS], FP16)
        qt_b = big.tile([64, QS], FP16)
        v_sb = big.tile([128, KB, HG, DH + 1], BF16)
        nc.vector.memset(v_sb[:, :, :, DH:DH + 1], 1.0)
        ot_a = big.tile([128, QS], FP16)
        ot_b = big.tile([64, QS], FP16)

        def xp_group(src, grp, dt, tag, nm):
            """xbar-transpose 512 rows x 768 cols of `src` (DRAM) into a
            [demb-chunk, 512] staging tile."""
            pool = vstagep if tag == "stage_v" else stagep
            st = pool.tile([128, DC, 512], dt, tag=tag, name=nm,
                           bufs=2 if tag == "stage_q" else None)
            rsl = slice(grp * 512, (grp + 1) * 512)
            for dc in range(DC):
                nc.sync.dma_start_transpose(
                    st[:, dc, :], src[rsl, dc * 128:(dc + 1) * 128])
            return st

        def proj_quad(stage, w_sb, b_sb, dst_a, dst_b, grp):
            """K/Q projection of one 512-col staging tile -> [hd, 512]
            columns of dst_a/dst_b (fp16)."""
            csl = slice(grp * 512, (grp + 1) * 512)
            pa = mixp.tile([128, 512], fp, tag="mix", name="pa")
            for dc in range(DC):
                nc.tensor.matmul(pa, lhsT=w_sb[:, dc, 0:128],
                                 rhs=stage[:, dc, :],
                                 start=(dc == 0), stop=False)
            nc.tensor.matmul(pa, lhsT=b_sb[0:1, 0:128], rhs=ones_16,
                             start=False, stop=True)
            nc.scalar.copy(dst_a[:, csl], pa)
            pb = mixp.tile([64, 512], fp, tag="mix", name="pb")
            for dc in range(DC):
                nc.tensor.matmul(pb, lhsT=w_sb[:, dc, 128:HD],
                                 rhs=stage[:, dc, :],
                                 start=(dc == 0), stop=False)
            nc.tensor.matmul(pb, lhsT=b_sb[0:1, 128:HD], rhs=ones_16,
                             start=False, stop=True)
            nc.scalar.copy(dst_b[0:64, csl], pb)

        def v_proj(v_st, kb):
            """Project one 128-key block of V (bf16) from its group's
            staging tile into v_sb."""
            ksl = slice((kb % 4) * 128, (kb % 4 + 1) * 128)
            pv = mixp.tile([128, HD], fp, tag="mix", name="pv")
            for dc in range(DC):
                nc.tensor.matmul(pv, lhsT=v_st[:, dc, ksl],
                                 rhs=wv_sb[:, dc, :],
                                 start=(dc == 0), stop=False)
            nc.tensor.matmul(pv, lhsT=ones_bf[0:1, 0:128], rhs=bv_sb,
                             start=False, stop=True)
            nc.vector.tensor_copy(
                v_sb[:, kb, :, 0:DH],
                pv.rearrange("p (h d) -> p h d", h=HG))

        class AttnIter:
            """One (head, query-chunk) attention pass; the conveyor emits
            its PV matmuls two steps behind the S^T/exp ones."""

            def __init__(self, h, qc, buffered=False):
                self.h, self.qc = h, qc
                if h < 2:
                    self.kt, self.qt, self.ot, self.r0 = \
                        kt_a, qt_a, ot_a, 64 * h
                else:
                    self.kt, self.qt, self.ot, self.r0 = kt_b, qt_b, ot_b, 0
                self.qsl = slice(qc * 512, (qc + 1) * 512)
                self.po = None
                self.buffered = buffered
                self.buf = []
                self.post = None

            def s_exp(self, kb2):
                pst = stp.tile([128, 1024], fp, tag="st", name="pst")
                for j in range(2):
                    kb = kb2 * 2 + j
                    nc.tensor.matmul(
                        pst[:, j * 512:(j + 1) * 512],
                        lhsT=self.kt[self.r0:self.r0 + 64,
                                     kb * 128:(kb + 1) * 128],
                        rhs=self.qt[self.r0:self.r0 + 64, self.qsl],
                        start=True, stop=True)
                pool = pt2p if self.buffered else ptp
                pt = pool.tile([128, 1024], BF16,
                               tag="pt2" if self.buffered else "pt",
                               name="pt")
                nc.scalar.activation(pt, pst, Exp, scale=SCALE)
                return pt

            def pv(self, pt, kb2):
                if self.po is None:
                    self.po = otp.tile([DH + 1, 512], fp, tag="ot",
                                       name="po")
                for j in range(2):
                    kb = kb2 * 2 + j
                    nc.tensor.matmul(self.po, lhsT=v_sb[:, kb, self.h, :],
                                     rhs=pt[:, j * 512:(j + 1) * 512],
                                     start=(kb == 0), stop=(kb == KB - 1))

            def epilogue(self):
                inv_z = smallp.tile([1, 512], fp, tag="invz", name="inv_z")
                nc.vector.reciprocal(inv_z, self.po[DH:DH + 1, :])
                bc_sb = smallp.tile([64, 512], fp, tag="bcs", name="bc_sb")
                nc.gpsimd.partition_broadcast(bc_sb, inv_z)
                nc.vector.tensor_mul(self.ot[self.r0:self.r0 + 64,
                                             self.qsl],
                                     self.po[0:DH, :], bc_sb)
                if self.post is not None:
                    self.post()

        # conveyor
        pending = []
        deferred = []
        replay = []

        def run_pending(depth=3):
            while len(pending) > depth:
                pending.pop(0)()

        def attn_step(it, kb2):
            pt = it.s_exp(kb2)
            if it.buffered:
                it.buf.append((pt, kb2))
                return

            def fin(it=it, pt=pt, kb2=kb2):
                it.pv(pt, kb2)
                if kb2 == KB // 2 - 1:
                    it.epilogue()
            pending.append(fin)
            run_pending()
            if replay:
                replay.pop(0)()
            elif deferred and kb2 % 4 == 3:
                deferred.pop(0)()

        def final_proj(qb):
            # PSUM from mixp (idle during attention) so finals never stall
            # the S^T double-buffering in stp
            qsl = slice(qb * 128, (qb + 1) * 128)
            o_sb = outp.tile([128, D], fp, tag="osb", name="o_sb")
            for n0, nsz in ((0, 512), (512, 256)):
                nsl = slice(n0, n0 + nsz)
                pf = mixp.tile([128, nsz], fp, tag="mix", name="pf")
                nc.tensor.matmul(pf, lhsT=ot_a[:, qsl],
                                 rhs=wo_a[:, nsl], start=True, stop=False)
                nc.tensor.matmul(pf, lhsT=ot_b[0:64, qsl],
                                 rhs=wo_b[0:64, nsl], start=False, stop=True)
                nc.vector.tensor_copy(o_sb[:, nsl], pf)
            nc.sync.dma_start(out=dr["out"][qsl, :], in_=o_sb)

        # ---- qc=0 queries, then the k/v loop with the qc=0 attention
        # iterations (heads 0/1 live, head 2 exp-buffered) interleaved ----
        q_st = xp_group(dr["q_in"], 0, FP16, "stage_q", "q_st")
        proj_quad(q_st, wq_sb, bq_sb, qt_a, qt_b, 0)
        it0 = AttnIter(0, 0)
        it1 = AttnIter(1, 0)
        it2 = AttnIter(2, 0, buffered=True)
        k_next = xp_group(dr["k_in"], 0, FP16, "stage", "k_st")
        v_next = xp_group(dr["v_in"], 0, BF16, "stage_v", "v_st")
        for g in range(NG):
            k_st, v_st = k_next, v_next
            if g + 1 < NG:
                k_next = xp_group(dr["k_in"], g + 1, FP16, "stage", "k_st")
                v_next = xp_group(dr["v_in"], g + 1, BF16, "stage_v",
                                  "v_st")
            proj_quad(k_st, wk_sb, bk_sb, kt_a, kt_b, g)
            if g == 0:
                q_st = xp_group(dr["q_in"], 1, FP16, "stage_q", "q_st")
            if g == 1:
                proj_quad(q_st, wq_sb, bq_sb, qt_a, qt_b, 1)
            for j in range(4):
                v_proj(v_st, g * 4 + j)
            for kb2 in (g * 2, g * 2 + 1):
                attn_step(it0, kb2)
                attn_step(it1, kb2)
                attn_step(it2, kb2)

        # head 2 of qc=0: its buffered PV chain replays one pair per
        # attention step of the next iteration
        def post0(qc=0):
            for qb in range(qc * 4, qc * 4 + 4):
                deferred.append(lambda qb=qb: final_proj(qb))
        it2.post = post0
        nbuf = len(it2.buf)
        for i, (pt, kb2) in enumerate(it2.buf):
            def unit(pt=pt, kb2=kb2, last=(i == nbuf - 1)):
                it2.pv(pt, kb2)
                if last:
                    it2.epilogue()
            replay.append(unit)

        # ---- remaining attention, query-chunk major ----
        for qc in range(1, QS // 512):
            for h in range(HG):
                if qc == 1 and h in (1, 2):
                    qg = h + 1
                    q_st = xp_group(dr["q_in"], qg, FP16, "stage_q",
                                    "q_st")
                    proj_quad(q_st, wq_sb, bq_sb, qt_a, qt_b, qg)
                it = AttnIter(h, qc)
                if h == HG - 1:
                    def post(qc=qc):
                        for qb in range(qc * 4, qc * 4 + 4):
                            deferred.append(lambda qb=qb: final_proj(qb))
                    it.post = post
                for kb2 in range(KB // 2):
                    attn_step(it, kb2)
        run_pending(depth=0)
        while replay:
            replay.pop(0)()
        while deferred:
            deferred.pop(0)()


def make_in_maps(q_in, k_in, v_in, W_q, b_q, W_k, b_k, W_v, b_v, W_o, b_o):
    c = np.ascontiguousarray
    bf = ml_dtypes.bfloat16
    f16 = np.float16
    q = np.asarray(q_in, np.float32)[0].astype(f16)
    k = np.asarray(k_in, np.float32)[0].astype(f16)
    v = np.asarray(v_in, np.float32)[0].astype(bf)
    in_maps = []
    for core in range(N_CORES):
        g, half = core // 2, core % 2
        hs = slice(HD * g, HD * (g + 1))
        in_maps.append({
            "q_in": c(q[QS * half:QS * (half + 1), :]),
            "k_in": c(k),
            "v_in": c(v),
            "w_q": c(np.asarray(W_q, np.float32)[:, hs].astype(f16)),
            "w_k": c(np.asarray(W_k, np.float32)[:, hs].astype(f16)),
            "w_v": c(np.asarray(W_v, np.float32)[:, hs].astype(bf)),
            "w_o": c(np.asarray(W_o, np.float32)[hs, :].astype(f16)),
            "b_q": c(np.asarray(b_q, np.float32)[hs].reshape(1, HD)
                     .astype(f16)),
            "b_k": c(np.asarray(b_k, np.float32)[hs].reshape(1, HD)
                     .astype(f16)),
            "b_v": c(np.asarray(b_v, np.float32)[hs].reshape(1, HD)
                     .astype(bf)),
        })
    return in_maps


def gather_out(results, b_o):
    """results: list of 8 per-core out arrays [QS, D] -> full [1, S, D]."""
    b_o = np.asarray(b_o, np.float32)
    full = np.zeros((1, S, D), np.float32)
    for half in range(2):
        acc = np.zeros((QS, D), np.float32)
        for g in range(G):
            acc += results[2 * g + half]
        full[0, QS * half:QS * (half + 1), :] = acc + b_o
    return full


_NC_CACHE = {}


def _get_nc():
    if "nc" not in _NC_CACHE:
        _NC_CACHE["nc"] = build_nc()
    return _NC_CACHE["nc"]


def kernel(q_in, k_in, v_in, W_q, b_q, W_k, b_k, W_v, b_v, W_o, b_o,
           _trace=False):
    nc = _get_nc()
    in_maps = make_in_maps(q_in, k_in, v_in, W_q, b_q, W_k, b_k, W_v, b_v,
                           W_o, b_o)
    res = run_bass_kernel_spmd(nc, in_maps, core_ids=list(range(N_CORES)),
                               trace=_trace)
    outs = [res.results[cr]["out"] for cr in range(N_CORES)]
    full = gather_out(outs, b_o)
    if _trace:
        kernel.last_results = res
    return full
